# revision 8
# baseline (speedup 1.0000x reference)
"""CaptionLoss (LSTM decode + cross-entropy) on 8 Trainium2 NeuronCores.

Strategy:
  - Host: build teacher-forced token ids, gather+transpose embedding rows,
    transpose weights into T-layout (feature on partition). All matmul
    operands quantized to fp8 e4m3 with x16 scaling (loss rel err ~1e-6,
    validated against the jax reference in fp64-combined golden model).
  - Device (one SPMD program on 8 cores, no collectives):
      * fused LSTM step: gates accumulate ih + hh + bias directly in PSUM
        (DoubleRow fp8 matmuls for both projections, bias added by a K=1
        ones-row matmul) -- no separate ih-precompute pass, no PSUM->SBUF
        copies, no DVE adds. tanh reads PSUM directly.
        All-tanh gate formulation (sigmoid(x)=(1+tanh(x/2))/2, c stored as
        2c) keeps every ACT op on one LUT table.
      * per-core 4000-wide vocab shard of fc_W: logits accumulate in
        2-bank PSUM tiles (1024 cols) so a single ACT Exp (scale fused,
        accum_out row-sum) covers two n-chunks -- halves the fixed
        ACT overhead per exp. fc matmuls fp8 DoubleRow; fc bias added
        in-PSUM by a K=1 ones-row matmul.
      * fc work is emitted interleaved with the LSTM steps (fills PE/ACT
        slack; exps deferred one step so they don't delay the gate chain).
  - Host: sum partial exp-sums across cores, target-logit dot from the
    exported hs, final log/sum reduction in f64.
"""

import numpy as np
import ml_dtypes as mld

B = 64
T = 50
TP1 = T + 1
R = TP1 * B          # 3264 sequence rows, t-major (r = t*B + b)
H = 512
E = 512
G = 4 * H            # 2048 gate rows
V = 32000
NC = 8
VS = V // NC         # 4000 vocab shard
KC = H // 128        # 4 contraction chunks
MC_G = G // 128      # 16 gate row chunks
MC_R = (R + 127) // 128   # 26 row chunks (last has 64 valid rows)
NQ = 4               # vocab shard split into 4 exp units (3x1024 + 928)
NV = 1024
SCL = 16.0           # fp8 operand scale; products carry 256x
START_IDX = 1
STOP_IDX = 2

_BUILT = None

import os
CFG_DBUDGET = int(os.environ.get("K_DBUDGET", "2"))
CFG_PSC = int(os.environ.get("K_PSC", "1"))
CFG_PSD = int(os.environ.get("K_PSD", "3"))
CFG_EOPRI = int(os.environ.get("K_EOPRI", "400"))


def _build():
    import concourse.bacc as bacc
    import concourse.mybir as mybir
    import concourse.tile as tile

    f32 = mybir.dt.float32
    f8 = mybir.dt.float8e4
    bf16 = mybir.dt.bfloat16
    DR = mybir.MatmulPerfMode.DoubleRow
    AF = mybir.ActivationFunctionType
    from concourse.alu_op_type import AluOpType

    nc = bacc.Bacc("TRN2", target_bir_lowering=False, debug=False,
                   num_devices=NC)

    # ---- DRAM I/O (fp8 operands pre-scaled x16 by host) --------------
    xTb_d = nc.dram_tensor("xTb", [H, B], f8, kind="ExternalInput")
    xTf_d = nc.dram_tensor("xTf", [H, B], f32, kind="ExternalInput")
    XT_d = nc.dram_tensor("XT", [E, R], f8, kind="ExternalInput")
    WihT_d = nc.dram_tensor("WihT", [E, G], f8, kind="ExternalInput")
    WhhT_d = nc.dram_tensor("WhhT", [H, G], f8, kind="ExternalInput")
    biasb_d = nc.dram_tensor("biasb", [1, G], bf16, kind="ExternalInput")
    fcWT_d = nc.dram_tensor("fcWT", [H, VS], f8, kind="ExternalInput")
    fcb_d = nc.dram_tensor("fcb", [1, VS], bf16, kind="ExternalInput")

    S_d = nc.dram_tensor("S", [128, MC_R], f32, kind="ExternalOutput")
    hs_d = nc.dram_tensor("hs", [128, KC * R], f8, kind="ExternalOutput")

    with tile.TileContext(nc) as tc:
        with (tc.tile_pool(name="glob", bufs=1) as gp,
              tc.tile_pool(name="xs", bufs=3) as xsp,
              tc.tile_pool(name="gs", bufs=2) as gsp,
              tc.tile_pool(name="fcs", bufs=3) as fsp,
              tc.tile_pool(name="psD", bufs=CFG_PSD, space="PSUM") as psD,
              tc.tile_pool(name="psC", bufs=CFG_PSC, space="PSUM") as psC):
            # ---- constants / state ----------------------------------
            WhhT = gp.tile([128, KC * G], f8)
            nc.sync.dma_start(
                out=WhhT[:, :].rearrange("p (k g) -> p k g", k=KC),
                in_=WhhT_d.ap().rearrange("(k p) g -> p k g", p=128))
            WihT = gp.tile([128, KC * G], f8)
            nc.sync.dma_start(
                out=WihT[:, :].rearrange("p (k g) -> p k g", k=KC),
                in_=WihT_d.ap().rearrange("(k p) g -> p k g", p=128))
            biasb = gp.tile([1, G], bf16)
            nc.sync.dma_start(out=biasb[:, :], in_=biasb_d[:, :])
            xTb = gp.tile([128, KC * B], f8)
            nc.sync.dma_start(
                out=xTb[:, :].rearrange("p (k b) -> p k b", k=KC),
                in_=xTb_d.ap().rearrange("(k p) b -> p k b", p=128))
            cT = gp.tile([128, KC * B], f32)
            nc.sync.dma_start(
                out=cT[:, :].rearrange("p (k b) -> p k b", k=KC),
                in_=xTf_d.ap().rearrange("(k p) b -> p k b", p=128))
            fcW = gp.tile([128, KC * VS], f8)
            fcb = gp.tile([1, VS], bf16)
            ones = gp.tile([1, 128], bf16)
            nc.gpsimd.memset(ones[:, :], 1.0)
            hsT = gp.tile([128, KC * R], f8)
            S_all = gp.tile([128, MC_R * NQ], f32)
            nc.vector.memset(S_all[:, :], 0.0)

            Wih3 = WihT[:, :].rearrange("p (k g) -> p k g", k=KC)
            Whh3 = WhhT[:, :].rearrange("p (k g) -> p k g", k=KC)
            xTb3 = xTb[:, :].rearrange("p (k b) -> p k b", k=KC)
            hs3 = hsT[:, :].rearrange("p (k r) -> p k r", k=KC)
            fcW3 = fcW[:, :].rearrange("p (k v) -> p k v", k=KC)

            # ---- XT chunk DMA (512 cols = 8 steps per chunk) --------
            n_chunks = []
            c0 = 0
            while c0 < R:
                w = min(512, R - c0)
                n_chunks.append((c0, w))
                c0 += w
            xt_tiles = {}

            def emit_B_dma(j):
                c0, w = n_chunks[j]
                xt = xsp.tile([128, KC * 512], f8, tag="xt")
                nc.sync.dma_start(
                    out=xt[:, 0:KC * w].rearrange("p (k n) -> p k n", k=KC),
                    in_=XT_d.ap().rearrange(
                        "(k p) n -> p k n", p=128)[:, :, c0:c0 + w])
                xt_tiles[j] = xt

            # ---- fused LSTM step ------------------------------------
            def emit_C(t, mid_act=None, end_act=None):
                c0, w = n_chunks[t // 8]
                xt3 = xt_tiles[t // 8][:, 0:KC * w].rearrange(
                    "p (k n) -> p k n", k=KC)
                toff = t * B - c0
                if t == 0:
                    hrhs = xTb3
                    hoff = 0
                else:
                    hrhs = hs3
                    hoff = (t - 1) * B
                # ih matmuls first (no dependency on the h chain), then
                # hh+bias; g/o half (m 8..15) first so its tanh overlaps
                # the i/f half's matmuls.
                ps1 = psC.tile([128, 512], f32, tag="ps1")
                ps0 = psC.tile([128, 512], f32, tag="ps0")
                order = list(range(8, 16)) + list(range(8))
                for m in order:
                    ps = ps0 if m < 8 else ps1
                    col = (m % 8) * B
                    for pr in range(2):
                        nc.tensor.matmul(
                            ps[:, col:col + B],
                            Wih3[:, 2 * pr:2 * pr + 2,
                                 m * 128:(m + 1) * 128],
                            xt3[:, 2 * pr:2 * pr + 2, toff:toff + B],
                            start=(pr == 0), stop=False, perf_mode=DR)
                for m in order:
                    ps = ps0 if m < 8 else ps1
                    col = (m % 8) * B
                    for pr in range(2):
                        nc.tensor.matmul(
                            ps[:, col:col + B],
                            Whh3[:, 2 * pr:2 * pr + 2,
                                 m * 128:(m + 1) * 128],
                            hrhs[:, 2 * pr:2 * pr + 2, hoff:hoff + B],
                            start=False, stop=False, perf_mode=DR)
                    nc.tensor.matmul(
                        ps[:, col:col + B],
                        biasb[:, m * 128:(m + 1) * 128],
                        ones[:, 0:B], start=False, stop=True)
                # all-tanh gates: tanh(x/512) of x256-scaled = tanh(g/2)
                s1 = gsp.tile([128, 512], f32, tag="s1")
                nc.scalar.activation(out=s1[:, :], in_=ps1[:, :],
                                     func=AF.Tanh, scale=1.0 / 512)
                tg = s1[:, 0:256]
                to = s1[:, 256:512]
                to8 = gsp.tile([128, 256], f32, tag="to8")
                nc.vector.tensor_scalar(
                    out=to8[:, :], in0=to, scalar1=8.0, scalar2=8.0,
                    op0=AluOpType.mult, op1=AluOpType.add)
                s0 = gsp.tile([128, 512], f32, tag="s0")
                nc.scalar.activation(out=s0[:, :], in_=ps0[:, :],
                                     func=AF.Tanh, scale=1.0 / 512)
                ti = s0[:, 0:256]
                tf = s0[:, 256:512]
                # deferred fc exp lands here: it fills the ACT gap while
                # DVE runs the c-state chain
                if mid_act is not None:
                    mid_act()
                # state is c2 = 2*c:  c2' = 0.5*(1+tf)*c2 + (1+ti)*tg
                u = gsp.tile([128, 256], f32, tag="u")
                nc.vector.scalar_tensor_tensor(
                    out=u[:, :], in0=tf, scalar=1.0,
                    in1=cT[:, :], op0=AluOpType.add, op1=AluOpType.mult)
                v = gsp.tile([128, 256], f32, tag="v")
                nc.vector.scalar_tensor_tensor(
                    out=v[:, :], in0=ti, scalar=1.0,
                    in1=tg, op0=AluOpType.add, op1=AluOpType.mult)
                nc.vector.scalar_tensor_tensor(
                    out=cT[:, :], in0=u[:, :], scalar=0.5,
                    in1=v[:, :], op0=AluOpType.mult, op1=AluOpType.add)
                th = gsp.tile([128, 256], f32, tag="th")
                nc.scalar.activation(out=th[:, :], in_=cT[:, :],
                                     func=AF.Tanh, scale=0.5)
                # h*16 = (8 + 8*to) * tanh(c), written as x16-scaled fp8
                hout = hs3[:, :, t * B:(t + 1) * B]
                nc.vector.tensor_tensor(out=hout, in0=to8[:, :],
                                        in1=th[:, :], op=AluOpType.mult)
                if end_act is not None:
                    end_act()

            # ---- phase D unit: rows [128m,128m+mw) x 1024 vocab -----
            def emit_D_mm(m, q):
                mw = min(128, R - m * 128)
                uw = min(NV, VS - q * NV)
                ps = psD.tile([128, NV], f32, tag="dps")
                for j in range((uw + 511) // 512):
                    n0 = q * NV + j * 512
                    nw = min(512, VS - n0)
                    for pr in range(2):
                        nc.tensor.matmul(
                            ps[0:mw, j * 512:j * 512 + nw],
                            hs3[:, 2 * pr:2 * pr + 2,
                                m * 128:m * 128 + mw],
                            fcW3[:, 2 * pr:2 * pr + 2, n0:n0 + nw],
                            start=(pr == 0), stop=False, perf_mode=DR)
                    nc.tensor.matmul(
                        ps[0:mw, j * 512:j * 512 + nw], ones[:, 0:mw],
                        fcb[:, n0:n0 + nw], start=False, stop=True)
                return (ps, m, q, mw, uw)

            def emit_D_exp(pend):
                # low scheduler priority: the exp is filler work -- never
                # let it delay the LSTM chain's tanh ops on ACT
                ps, m, q, mw, uw = pend
                eo = fsp.tile([128, NV], bf16, tag="eo")
                with tc.high_priority(offset=-CFG_EOPRI):
                    nc.scalar.activation(
                        out=eo[0:mw, 0:uw], in_=ps[0:mw, 0:uw], func=AF.Exp,
                        scale=1.0 / 256,
                        accum_out=S_all[0:mw, m * NQ + q:m * NQ + q + 1])

            # ---- interleaved emission -------------------------------
            emit_B_dma(0)
            emit_B_dma(1)
            for k in range(KC):
                nc.sync.dma_start(
                    out=fcW[:, k * VS:(k + 1) * VS],
                    in_=fcWT_d[k * 128:(k + 1) * 128, :])
            nc.sync.dma_start(out=fcb[:, :], in_=fcb_d[:, :])
            d_queue = [(m, q) for m in range(MC_R) for q in range(NQ)]
            d_next = 0
            pending = []
            for t in range(TP1):
                # D matmul units first: PE runs them during the previous
                # step's tanh/c chain; their exps are deferred into this
                # step's ACT gaps (mid_act / end_act)
                m_ready = (t - 2) // 2 if t >= 2 else -1
                new_units = []
                while len(new_units) < CFG_DBUDGET and d_next < len(d_queue):
                    m, q = d_queue[d_next]
                    if m > m_ready:
                        break
                    new_units.append(emit_D_mm(m, q))
                    d_next += 1
                if t % 8 == 0:
                    j = t // 8 + 2
                    if j < len(n_chunks):
                        emit_B_dma(j)
                exps = list(pending)
                pending = new_units

                def mid():
                    if exps:
                        emit_D_exp(exps.pop(0))

                def end():
                    while exps:
                        emit_D_exp(exps.pop(0))

                emit_C(t, mid_act=mid, end_act=end)
            while d_next < len(d_queue) or pending:
                for p in pending:
                    emit_D_exp(p)
                pending = []
                n_emit = 0
                while n_emit < CFG_DBUDGET and d_next < len(d_queue):
                    m, q = d_queue[d_next]
                    pending.append(emit_D_mm(m, q))
                    d_next += 1
                    n_emit += 1

            nc.sync.dma_start(out=hs_d[:, :], in_=hsT[:, :])
            S_fin = gp.tile([128, MC_R], f32)
            nc.vector.reduce_sum(
                out=S_fin[:, :],
                in_=S_all[:, :].rearrange("p (m n) -> p m n", n=NQ),
                axis=mybir.AxisListType.X)
            nc.sync.dma_start(out=S_d[:, :], in_=S_fin[:, :])

    nc.compile()
    return nc


def _get_built():
    global _BUILT
    if _BUILT is None:
        _BUILT = _build()
    return _BUILT


def _q8(a):
    return np.clip(a, -240.0, 240.0).astype(mld.float8_e4m3)


def prep_in_maps(x, labels, emb, W_ih, W_hh, b_ih, b_hh, fc_W, fc_b):
    lab = labels.astype(np.int64)
    inputs = np.concatenate(
        [np.full((B, 1), START_IDX, np.int64), lab], axis=1)      # [B, 51]
    targets = np.concatenate(
        [lab, np.full((B, 1), STOP_IDX, np.int64)], axis=1)       # [B, 51]
    idx = inputs.T.reshape(-1)      # [3264] t-major
    tgt = targets.T.reshape(-1)

    # unified tanh(x/512): g-gate rows (the tanh gate) carry half scale
    gsc = np.ones((G,), np.float32)
    gsc[2 * H:3 * H] = 2.0
    base = {
        "xTb": _q8(np.ascontiguousarray(x.T) * SCL),
        "xTf": (np.ascontiguousarray(x.T) * 2.0).astype(np.float32),
        "XT": _q8(np.ascontiguousarray(emb[idx].T) * SCL),
        "WihT": _q8(np.ascontiguousarray((W_ih * gsc[:, None]).T) * SCL),
        "WhhT": _q8(np.ascontiguousarray((W_hh * gsc[:, None]).T) * SCL),
        "biasb": ((b_ih + b_hh) * gsc * 256.0)[None, :].astype(mld.bfloat16),
    }
    in_maps = []
    for c in range(NC):
        sh = slice(c * VS, (c + 1) * VS)
        in_maps.append(dict(
            base,
            fcWT=_q8(np.ascontiguousarray(fc_W[sh].T) * SCL),
            fcb=(fc_b[sh][None, :] * 256.0).astype(mld.bfloat16)))
    return in_maps, tgt


def combine(results, tgt, fc_W, fc_b):
    S_rows = np.zeros(R, np.float64)
    for c in range(NC):
        S_rows += np.asarray(
            results[c]["S"], np.float64).T.reshape(-1)[:R]
    hs0 = np.asarray(results[0]["hs"]).astype(np.float32) / SCL   # [128, 4*R]
    hs_rows = hs0.reshape(128, KC, R).transpose(2, 1, 0).reshape(R, H)
    Wt = fc_W[tgt].astype(mld.bfloat16).astype(np.float32)        # [3264, 512]
    tgt_dot = (hs_rows * Wt).sum(1, dtype=np.float32)
    nll = np.log(S_rows) - (tgt_dot.astype(np.float64) + fc_b[tgt])
    return np.float32(nll.sum() / B)


def kernel(x, labels, emb, W_ih, W_hh, b_ih, b_hh, fc_W, fc_b):
    from concourse.bass_utils import run_bass_kernel_spmd

    x = np.asarray(x, np.float32)
    emb = np.asarray(emb, np.float32)
    W_ih = np.asarray(W_ih, np.float32)
    W_hh = np.asarray(W_hh, np.float32)
    b_ih = np.asarray(b_ih, np.float32)
    b_hh = np.asarray(b_hh, np.float32)
    fc_W = np.asarray(fc_W, np.float32)
    fc_b = np.asarray(fc_b, np.float32)

    in_maps, tgt = prep_in_maps(x, np.asarray(labels), emb, W_ih, W_hh,
                                b_ih, b_hh, fc_W, fc_b)
    nc = _get_built()
    res = run_bass_kernel_spmd(nc, in_maps, core_ids=list(range(NC)))
    return combine(res.results, tgt, fc_W, fc_b)


# revision 13
# speedup vs baseline: 1.0471x; 1.0471x over previous
"""CaptionLoss (LSTM decode + cross-entropy) on 8 Trainium2 NeuronCores.

Strategy:
  - Host: build teacher-forced token ids, gather+transpose embedding rows,
    transpose weights into T-layout (feature on partition). All matmul
    operands quantized to fp8 e4m3 with x16 scaling (loss rel err ~1e-6,
    validated against the jax reference in fp64-combined golden model).
  - Device (one SPMD program on 8 cores, no collectives):
      * fused LSTM step: gates accumulate ih + hh + bias directly in PSUM
        (DoubleRow fp8 matmuls for both projections, bias added by a K=1
        ones-row matmul) -- no separate ih-precompute pass, no PSUM->SBUF
        copies, no DVE adds. tanh reads PSUM directly.
        All-tanh gate formulation (sigmoid(x)=(1+tanh(x/2))/2, c stored as
        2c) keeps every ACT op on one LUT table.
      * per-core 4000-wide vocab shard of fc_W: logits accumulate in
        2-bank PSUM tiles (1024 cols) so a single ACT Exp (scale fused,
        accum_out row-sum) covers two n-chunks -- halves the fixed
        ACT overhead per exp. fc matmuls fp8 DoubleRow; fc bias added
        in-PSUM by a K=1 ones-row matmul.
      * fc work is emitted interleaved with the LSTM steps (fills PE/ACT
        slack; exps deferred one step so they don't delay the gate chain).
  - Host: sum partial exp-sums across cores, target-logit dot from the
    exported hs, final log/sum reduction in f64.
"""

import numpy as np
import ml_dtypes as mld

B = 64
T = 50
TP1 = T + 1
R = TP1 * B          # 3264 sequence rows, t-major (r = t*B + b)
H = 512
E = 512
G = 4 * H            # 2048 gate rows
V = 32000
NC = 8
VS = V // NC         # 4000 vocab shard
KC = H // 128        # 4 contraction chunks
MC_G = G // 128      # 16 gate row chunks
MC_R = (R + 127) // 128   # 26 row chunks (last has 64 valid rows)
NQ = 4               # vocab shard split into 4 exp units (3x1024 + 928)
NV = 1024
SCL = 16.0           # fp8 operand scale; products carry 256x
START_IDX = 1
STOP_IDX = 2

_BUILT = None

import os
CFG_DBUDGET = int(os.environ.get("K_DBUDGET", "2"))
CFG_PSC = int(os.environ.get("K_PSC", "1"))
CFG_PSD = int(os.environ.get("K_PSD", "3"))
CFG_EOPRI = int(os.environ.get("K_EOPRI", "400"))


def _build():
    import concourse.bacc as bacc
    import concourse.mybir as mybir
    import concourse.tile as tile

    f32 = mybir.dt.float32
    f8 = mybir.dt.float8e4
    bf16 = mybir.dt.bfloat16
    DR = mybir.MatmulPerfMode.DoubleRow
    AF = mybir.ActivationFunctionType
    from concourse.alu_op_type import AluOpType

    nc = bacc.Bacc("TRN2", target_bir_lowering=False, debug=False,
                   num_devices=NC)

    # ---- DRAM I/O (fp8 operands pre-scaled x16 by host) --------------
    xTb_d = nc.dram_tensor("xTb", [H, B], f8, kind="ExternalInput")
    xTf_d = nc.dram_tensor("xTf", [H, B], f32, kind="ExternalInput")
    XT_d = nc.dram_tensor("XT", [E, R], f8, kind="ExternalInput")
    WihT_d = nc.dram_tensor("WihT", [E, G], f8, kind="ExternalInput")
    WhhT_d = nc.dram_tensor("WhhT", [H, G], f8, kind="ExternalInput")
    biasb_d = nc.dram_tensor("biasb", [1, G], bf16, kind="ExternalInput")
    fcWT_d = nc.dram_tensor("fcWT", [H, VS], f8, kind="ExternalInput")
    fcb_d = nc.dram_tensor("fcb", [1, VS], bf16, kind="ExternalInput")

    S_d = nc.dram_tensor("S", [128, MC_R], f32, kind="ExternalOutput")
    hs_d = nc.dram_tensor("hs", [128, KC * R], f8, kind="ExternalOutput")

    with tile.TileContext(nc) as tc:
        with (tc.tile_pool(name="glob", bufs=1) as gp,
              tc.tile_pool(name="xs", bufs=3) as xsp,
              tc.tile_pool(name="gs", bufs=2) as gsp,
              tc.tile_pool(name="fcs", bufs=3) as fsp,
              tc.tile_pool(name="psD", bufs=CFG_PSD, space="PSUM") as psD,
              tc.tile_pool(name="psC", bufs=CFG_PSC, space="PSUM") as psC):
            # ---- constants / state ----------------------------------
            WhhT = gp.tile([128, KC * G], f8)
            nc.sync.dma_start(
                out=WhhT[:, :].rearrange("p (k g) -> p k g", k=KC),
                in_=WhhT_d.ap().rearrange("(k p) g -> p k g", p=128))
            WihT = gp.tile([128, KC * G], f8)
            nc.sync.dma_start(
                out=WihT[:, :].rearrange("p (k g) -> p k g", k=KC),
                in_=WihT_d.ap().rearrange("(k p) g -> p k g", p=128))
            biasb = gp.tile([1, G], bf16)
            nc.sync.dma_start(out=biasb[:, :], in_=biasb_d[:, :])
            xTb = gp.tile([128, KC * B], f8)
            nc.sync.dma_start(
                out=xTb[:, :].rearrange("p (k b) -> p k b", k=KC),
                in_=xTb_d.ap().rearrange("(k p) b -> p k b", p=128))
            cT = gp.tile([128, KC * B], f32)
            nc.sync.dma_start(
                out=cT[:, :].rearrange("p (k b) -> p k b", k=KC),
                in_=xTf_d.ap().rearrange("(k p) b -> p k b", p=128))
            fcW = gp.tile([128, KC * VS], f8)
            fcb = gp.tile([1, VS], bf16)
            ones = gp.tile([1, 128], bf16)
            nc.gpsimd.memset(ones[:, :], 1.0)
            hsT = gp.tile([128, KC * R], f8)
            S_all = gp.tile([128, MC_R * NQ], f32)
            nc.vector.memset(S_all[:, :], 0.0)

            Wih3 = WihT[:, :].rearrange("p (k g) -> p k g", k=KC)
            Whh3 = WhhT[:, :].rearrange("p (k g) -> p k g", k=KC)
            xTb3 = xTb[:, :].rearrange("p (k b) -> p k b", k=KC)
            # hs stored r-major (col = r*KC + k): step t's write interval
            # [t*B*KC, (t+1)*B*KC) is disjoint from the fc matmuls' reads
            # of past rows, so the dependency tracker doesn't serialize
            # fc work behind every LSTM step.
            hs3 = hsT[:, :].rearrange("p (r k) -> p k r", k=KC)
            fcW3 = fcW[:, :].rearrange("p (k v) -> p k v", k=KC)
            # k-major staging of hs for the fc matmuls' stationary operand
            # (dual-fp8 Ldweights needs contiguous rows). Chunk m holds
            # rows [128m,128m+128) at cols [512m,512m+512), k-major, so
            # each step's copy stays inside its own chunk -- no false
            # deps against fc reads of older chunks.
            hsTk = gp.tile([128, MC_R * 512], f8)
            hsTk4 = hsTk[:, :].rearrange(
                "p (m k r) -> p m k r", m=MC_R, k=KC)

            # ---- XT chunk DMA (512 cols = 8 steps per chunk) --------
            n_chunks = []
            c0 = 0
            while c0 < R:
                w = min(512, R - c0)
                n_chunks.append((c0, w))
                c0 += w
            xt_tiles = {}

            def emit_B_dma(j):
                c0, w = n_chunks[j]
                xt = xsp.tile([128, KC * 512], f8, tag="xt")
                nc.sync.dma_start(
                    out=xt[:, 0:KC * w].rearrange("p (k n) -> p k n", k=KC),
                    in_=XT_d.ap().rearrange(
                        "(k p) n -> p k n", p=128)[:, :, c0:c0 + w])
                xt_tiles[j] = xt

            # ---- fused LSTM step ------------------------------------
            def emit_C(t, mid_act=None, end_act=None):
                c0, w = n_chunks[t // 8]
                xt3 = xt_tiles[t // 8][:, 0:KC * w].rearrange(
                    "p (k n) -> p k n", k=KC)
                toff = t * B - c0
                if t == 0:
                    hrhs = xTb3
                    hoff = 0
                else:
                    hrhs = hs3
                    hoff = (t - 1) * B
                # ih matmuls first (no dependency on the h chain), then
                # hh+bias; g/o half (m 8..15) first so its tanh overlaps
                # the i/f half's matmuls.
                ps1 = psC.tile([128, 512], f32, tag="ps1")
                ps0 = psC.tile([128, 512], f32, tag="ps0")
                order = list(range(8, 16)) + list(range(8))
                for m in order:
                    ps = ps0 if m < 8 else ps1
                    col = (m % 8) * B
                    for pr in range(2):
                        nc.tensor.matmul(
                            ps[:, col:col + B],
                            Wih3[:, 2 * pr:2 * pr + 2,
                                 m * 128:(m + 1) * 128],
                            xt3[:, 2 * pr:2 * pr + 2, toff:toff + B],
                            start=(pr == 0), stop=False, perf_mode=DR)
                for m in order:
                    ps = ps0 if m < 8 else ps1
                    col = (m % 8) * B
                    for pr in range(2):
                        nc.tensor.matmul(
                            ps[:, col:col + B],
                            Whh3[:, 2 * pr:2 * pr + 2,
                                 m * 128:(m + 1) * 128],
                            hrhs[:, 2 * pr:2 * pr + 2, hoff:hoff + B],
                            start=False, stop=False, perf_mode=DR)
                    nc.tensor.matmul(
                        ps[:, col:col + B],
                        biasb[:, m * 128:(m + 1) * 128],
                        ones[:, 0:B], start=False, stop=True)
                # all-tanh gates: tanh(x/512) of x256-scaled = tanh(g/2)
                s1 = gsp.tile([128, 512], f32, tag="s1")
                nc.scalar.activation(out=s1[:, :], in_=ps1[:, :],
                                     func=AF.Tanh, scale=1.0 / 512)
                tg = s1[:, 0:256]
                to = s1[:, 256:512]
                to8 = gsp.tile([128, 256], f32, tag="to8")
                nc.vector.tensor_scalar(
                    out=to8[:, :], in0=to, scalar1=8.0, scalar2=8.0,
                    op0=AluOpType.mult, op1=AluOpType.add)
                s0 = gsp.tile([128, 512], f32, tag="s0")
                nc.scalar.activation(out=s0[:, :], in_=ps0[:, :],
                                     func=AF.Tanh, scale=1.0 / 512)
                ti = s0[:, 0:256]
                tf = s0[:, 256:512]
                # deferred fc exp lands here: it fills the ACT gap while
                # DVE runs the c-state chain
                if mid_act is not None:
                    mid_act()
                # state is c2 = 2*c:  c2' = 0.5*(1+tf)*c2 + (1+ti)*tg
                u = gsp.tile([128, 256], f32, tag="u")
                nc.vector.scalar_tensor_tensor(
                    out=u[:, :], in0=tf, scalar=1.0,
                    in1=cT[:, :], op0=AluOpType.add, op1=AluOpType.mult)
                v = gsp.tile([128, 256], f32, tag="v")
                nc.vector.scalar_tensor_tensor(
                    out=v[:, :], in0=ti, scalar=1.0,
                    in1=tg, op0=AluOpType.add, op1=AluOpType.mult)
                nc.vector.scalar_tensor_tensor(
                    out=cT[:, :], in0=u[:, :], scalar=0.5,
                    in1=v[:, :], op0=AluOpType.mult, op1=AluOpType.add)
                th = gsp.tile([128, 256], f32, tag="th")
                nc.scalar.activation(out=th[:, :], in_=cT[:, :],
                                     func=AF.Tanh, scale=0.5)
                # h*16 = (8 + 8*to) * tanh(c), written as x16-scaled fp8
                hout = hs3[:, :, t * B:(t + 1) * B]
                nc.vector.tensor_tensor(out=hout, in0=to8[:, :],
                                        in1=th[:, :], op=AluOpType.mult)
                if end_act is not None:
                    end_act()
                # stage this step's h into the k-major fc layout (gpsimd:
                # off the critical chain, Pool engine is otherwise idle)
                half = (t % 2) * B
                nc.gpsimd.tensor_copy(
                    out=hsTk4[:, t // 2, :, half:half + B],
                    in_=hout)

            # ---- phase D unit: rows [128m,128m+mw) x 1024 vocab -----
            def emit_D_mm(m, q):
                mw = min(128, R - m * 128)
                uw = min(NV, VS - q * NV)
                ps = psD.tile([128, NV], f32, tag="dps")
                for j in range((uw + 511) // 512):
                    n0 = q * NV + j * 512
                    nw = min(512, VS - n0)
                    for pr in range(2):
                        nc.tensor.matmul(
                            ps[0:mw, j * 512:j * 512 + nw],
                            hsTk4[:, m, 2 * pr:2 * pr + 2, 0:mw],
                            fcW3[:, 2 * pr:2 * pr + 2, n0:n0 + nw],
                            start=(pr == 0), stop=False, perf_mode=DR)
                    nc.tensor.matmul(
                        ps[0:mw, j * 512:j * 512 + nw], ones[:, 0:mw],
                        fcb[:, n0:n0 + nw], start=False, stop=True)
                return (ps, m, q, mw, uw)

            def emit_D_exp(pend):
                # low scheduler priority: the exp is filler work -- never
                # let it delay the LSTM chain's tanh ops on ACT
                ps, m, q, mw, uw = pend
                eo = fsp.tile([128, NV], bf16, tag="eo")
                with tc.high_priority(offset=-CFG_EOPRI):
                    nc.scalar.activation(
                        out=eo[0:mw, 0:uw], in_=ps[0:mw, 0:uw], func=AF.Exp,
                        scale=1.0 / 256,
                        accum_out=S_all[0:mw, m * NQ + q:m * NQ + q + 1])

            # ---- interleaved emission -------------------------------
            emit_B_dma(0)
            emit_B_dma(1)
            for k in range(KC):
                nc.sync.dma_start(
                    out=fcW[:, k * VS:(k + 1) * VS],
                    in_=fcWT_d[k * 128:(k + 1) * 128, :])
            nc.sync.dma_start(out=fcb[:, :], in_=fcb_d[:, :])
            d_queue = [(m, q) for m in range(MC_R) for q in range(NQ)]
            d_next = 0
            pending = []
            for t in range(TP1):
                # D matmul units first: PE runs them during the previous
                # step's tanh/c chain; their exps are deferred into this
                # step's ACT gaps (mid_act / end_act)
                m_ready = (t - 2) // 2 if t >= 2 else -1
                new_units = []
                while len(new_units) < CFG_DBUDGET and d_next < len(d_queue):
                    m, q = d_queue[d_next]
                    if m > m_ready:
                        break
                    new_units.append(emit_D_mm(m, q))
                    d_next += 1
                if t % 8 == 0:
                    j = t // 8 + 2
                    if j < len(n_chunks):
                        emit_B_dma(j)
                exps = list(pending)
                pending = new_units

                def mid():
                    if exps:
                        emit_D_exp(exps.pop(0))

                def end():
                    while exps:
                        emit_D_exp(exps.pop(0))

                emit_C(t, mid_act=mid, end_act=end)
            while d_next < len(d_queue) or pending:
                for p in pending:
                    emit_D_exp(p)
                pending = []
                n_emit = 0
                while n_emit < CFG_DBUDGET and d_next < len(d_queue):
                    m, q = d_queue[d_next]
                    pending.append(emit_D_mm(m, q))
                    d_next += 1
                    n_emit += 1

            nc.sync.dma_start(out=hs_d[:, :], in_=hsT[:, :])
            S_fin = gp.tile([128, MC_R], f32)
            nc.vector.reduce_sum(
                out=S_fin[:, :],
                in_=S_all[:, :].rearrange("p (m n) -> p m n", n=NQ),
                axis=mybir.AxisListType.X)
            nc.sync.dma_start(out=S_d[:, :], in_=S_fin[:, :])

    nc.compile()
    return nc


def _get_built():
    global _BUILT
    if _BUILT is None:
        _BUILT = _build()
    return _BUILT


def _q8(a):
    return np.clip(a, -240.0, 240.0).astype(mld.float8_e4m3)


def prep_in_maps(x, labels, emb, W_ih, W_hh, b_ih, b_hh, fc_W, fc_b):
    lab = labels.astype(np.int64)
    inputs = np.concatenate(
        [np.full((B, 1), START_IDX, np.int64), lab], axis=1)      # [B, 51]
    targets = np.concatenate(
        [lab, np.full((B, 1), STOP_IDX, np.int64)], axis=1)       # [B, 51]
    idx = inputs.T.reshape(-1)      # [3264] t-major
    tgt = targets.T.reshape(-1)

    # unified tanh(x/512): g-gate rows (the tanh gate) carry half scale
    gsc = np.ones((G,), np.float32)
    gsc[2 * H:3 * H] = 2.0
    base = {
        "xTb": _q8(np.ascontiguousarray(x.T) * SCL),
        "xTf": (np.ascontiguousarray(x.T) * 2.0).astype(np.float32),
        "XT": _q8(np.ascontiguousarray(emb[idx].T) * SCL),
        "WihT": _q8(np.ascontiguousarray((W_ih * gsc[:, None]).T) * SCL),
        "WhhT": _q8(np.ascontiguousarray((W_hh * gsc[:, None]).T) * SCL),
        "biasb": ((b_ih + b_hh) * gsc * 256.0)[None, :].astype(mld.bfloat16),
    }
    in_maps = []
    for c in range(NC):
        sh = slice(c * VS, (c + 1) * VS)
        in_maps.append(dict(
            base,
            fcWT=_q8(np.ascontiguousarray(fc_W[sh].T) * SCL),
            fcb=(fc_b[sh][None, :] * 256.0).astype(mld.bfloat16)))
    return in_maps, tgt


def combine(results, tgt, fc_W, fc_b):
    S_rows = np.zeros(R, np.float64)
    for c in range(NC):
        S_rows += np.asarray(
            results[c]["S"], np.float64).T.reshape(-1)[:R]
    hs0 = np.asarray(results[0]["hs"]).astype(np.float32) / SCL   # [128, R*4]
    hs_rows = hs0.reshape(128, R, KC).transpose(1, 2, 0).reshape(R, H)
    Wt = fc_W[tgt].astype(mld.bfloat16).astype(np.float32)        # [3264, 512]
    tgt_dot = (hs_rows * Wt).sum(1, dtype=np.float32)
    nll = np.log(S_rows) - (tgt_dot.astype(np.float64) + fc_b[tgt])
    return np.float32(nll.sum() / B)


def kernel(x, labels, emb, W_ih, W_hh, b_ih, b_hh, fc_W, fc_b):
    from concourse.bass_utils import run_bass_kernel_spmd

    x = np.asarray(x, np.float32)
    emb = np.asarray(emb, np.float32)
    W_ih = np.asarray(W_ih, np.float32)
    W_hh = np.asarray(W_hh, np.float32)
    b_ih = np.asarray(b_ih, np.float32)
    b_hh = np.asarray(b_hh, np.float32)
    fc_W = np.asarray(fc_W, np.float32)
    fc_b = np.asarray(fc_b, np.float32)

    in_maps, tgt = prep_in_maps(x, np.asarray(labels), emb, W_ih, W_hh,
                                b_ih, b_hh, fc_W, fc_b)
    nc = _get_built()
    res = run_bass_kernel_spmd(nc, in_maps, core_ids=list(range(NC)))
    return combine(res.results, tgt, fc_W, fc_b)


# revision 14
# speedup vs baseline: 1.2590x; 1.2023x over previous
"""CaptionLoss (LSTM decode + cross-entropy) on 8 Trainium2 NeuronCores.

Strategy:
  - Host: build teacher-forced token ids, gather+transpose embedding rows,
    transpose weights into T-layout (feature on partition). All matmul
    operands quantized to fp8 e4m3 with x16 scaling (loss rel err ~1e-6,
    validated against the jax reference in fp64-combined golden model).
  - Device (one SPMD program on 8 cores, no collectives):
      * fused LSTM step: gates accumulate ih + hh + bias directly in PSUM
        (DoubleRow fp8 matmuls for both projections, bias added by a K=1
        ones-row matmul) -- no separate ih-precompute pass, no PSUM->SBUF
        copies, no DVE adds. tanh reads PSUM directly.
        All-tanh gate formulation (sigmoid(x)=(1+tanh(x/2))/2, c stored as
        2c) keeps every ACT op on one LUT table.
      * per-core 4000-wide vocab shard of fc_W: logits accumulate in
        2-bank PSUM tiles (1024 cols) so a single ACT Exp (scale fused,
        accum_out row-sum) covers two n-chunks -- halves the fixed
        ACT overhead per exp. fc matmuls fp8 DoubleRow; fc bias added
        in-PSUM by a K=1 ones-row matmul.
      * fc work is emitted interleaved with the LSTM steps (fills PE/ACT
        slack; exps deferred one step so they don't delay the gate chain).
  - Host: sum partial exp-sums across cores, target-logit dot from the
    exported hs, final log/sum reduction in f64.
"""

import numpy as np
import ml_dtypes as mld

B = 64
T = 50
TP1 = T + 1
R = TP1 * B          # 3264 sequence rows, t-major (r = t*B + b)
H = 512
E = 512
G = 4 * H            # 2048 gate rows
V = 32000
NC = 8
VS = V // NC         # 4000 vocab shard
KC = H // 128        # 4 contraction chunks
MC_G = G // 128      # 16 gate row chunks
MC_R = (R + 127) // 128   # 26 row chunks (last has 64 valid rows)
NQ = 4               # vocab shard split into 4 exp units (3x1024 + 928)
NV = 1024
SCL = 16.0           # fp8 operand scale; products carry 256x
START_IDX = 1
STOP_IDX = 2

_BUILT = None

import os
CFG_DBUDGET = int(os.environ.get("K_DBUDGET", "2"))
CFG_PSC = int(os.environ.get("K_PSC", "1"))
CFG_PSD = int(os.environ.get("K_PSD", "3"))
CFG_EOPRI = int(os.environ.get("K_EOPRI", "400"))


def _build():
    import concourse.bacc as bacc
    import concourse.mybir as mybir
    import concourse.tile as tile

    f32 = mybir.dt.float32
    f8 = mybir.dt.float8e4
    bf16 = mybir.dt.bfloat16
    DR = mybir.MatmulPerfMode.DoubleRow
    AF = mybir.ActivationFunctionType
    from concourse.alu_op_type import AluOpType

    nc = bacc.Bacc("TRN2", target_bir_lowering=False, debug=False,
                   num_devices=NC)

    # ---- DRAM I/O (fp8 operands pre-scaled x16 by host) --------------
    xTb_d = nc.dram_tensor("xTb", [H, B], f8, kind="ExternalInput")
    xTf_d = nc.dram_tensor("xTf", [H, B], f32, kind="ExternalInput")
    XT_d = nc.dram_tensor("XT", [E, R], f8, kind="ExternalInput")
    WihT_d = nc.dram_tensor("WihT", [E, G], f8, kind="ExternalInput")
    WhhT_d = nc.dram_tensor("WhhT", [H, G], f8, kind="ExternalInput")
    biasb_d = nc.dram_tensor("biasb", [1, G], bf16, kind="ExternalInput")
    fcWT_d = nc.dram_tensor("fcWT", [H, VS], f8, kind="ExternalInput")
    fcb_d = nc.dram_tensor("fcb", [1, VS], bf16, kind="ExternalInput")

    S_d = nc.dram_tensor("S", [128, MC_R], f32, kind="ExternalOutput")
    hs_d = nc.dram_tensor("hs", [128, KC * R], f8, kind="ExternalOutput")

    with tile.TileContext(nc) as tc:
        with (tc.tile_pool(name="glob", bufs=1) as gp,
              tc.tile_pool(name="xs", bufs=3) as xsp,
              tc.tile_pool(name="gs", bufs=2) as gsp,
              tc.tile_pool(name="fcs", bufs=3) as fsp,
              tc.tile_pool(name="psD", bufs=CFG_PSD, space="PSUM") as psD,
              tc.tile_pool(name="psC", bufs=CFG_PSC, space="PSUM") as psC):
            # ---- constants / state ----------------------------------
            WhhT = gp.tile([128, KC * G], f8)
            nc.sync.dma_start(
                out=WhhT[:, :].rearrange("p (k g) -> p k g", k=KC),
                in_=WhhT_d.ap().rearrange("(k p) g -> p k g", p=128))
            WihT = gp.tile([128, KC * G], f8)
            nc.sync.dma_start(
                out=WihT[:, :].rearrange("p (k g) -> p k g", k=KC),
                in_=WihT_d.ap().rearrange("(k p) g -> p k g", p=128))
            biasb = gp.tile([1, G], bf16)
            nc.sync.dma_start(out=biasb[:, :], in_=biasb_d[:, :])
            xTb = gp.tile([128, KC * B], f8)
            nc.sync.dma_start(
                out=xTb[:, :].rearrange("p (k b) -> p k b", k=KC),
                in_=xTb_d.ap().rearrange("(k p) b -> p k b", p=128))
            cT = gp.tile([128, KC * B], f32)
            nc.sync.dma_start(
                out=cT[:, :].rearrange("p (k b) -> p k b", k=KC),
                in_=xTf_d.ap().rearrange("(k p) b -> p k b", p=128))
            fcW = gp.tile([128, KC * VS], f8)
            fcb = gp.tile([1, VS], bf16)
            ones = gp.tile([1, 128], bf16)
            nc.gpsimd.memset(ones[:, :], 1.0)
            hsT = gp.tile([128, KC * R], f8)
            S_all = gp.tile([128, MC_R * NQ], f32)
            nc.vector.memset(S_all[:, :], 0.0)

            Wih3 = WihT[:, :].rearrange("p (k g) -> p k g", k=KC)
            Whh3 = WhhT[:, :].rearrange("p (k g) -> p k g", k=KC)
            xTb3 = xTb[:, :].rearrange("p (k b) -> p k b", k=KC)
            # hs stored r-major (col = r*KC + k): step t's write interval
            # [t*B*KC, (t+1)*B*KC) is disjoint from the fc matmuls' reads
            # of past rows, so the dependency tracker doesn't serialize
            # fc work behind every LSTM step.
            hs3 = hsT[:, :].rearrange("p (r k) -> p k r", k=KC)
            fcW3 = fcW[:, :].rearrange("p (k v) -> p k v", k=KC)
            # k-major staging of hs for the fc matmuls' stationary operand
            # (dual-fp8 Ldweights needs contiguous rows). Chunk m holds
            # rows [128m,128m+128) at cols [512m,512m+512), k-major, so
            # each step's copy stays inside its own chunk -- no false
            # deps against fc reads of older chunks.
            hsTk = gp.tile([128, MC_R * 512], f8)
            hsTk4 = hsTk[:, :].rearrange(
                "p (m k r) -> p m k r", m=MC_R, k=KC)

            # ---- XT chunk DMA (512 cols = 8 steps per chunk) --------
            n_chunks = []
            c0 = 0
            while c0 < R:
                w = min(512, R - c0)
                n_chunks.append((c0, w))
                c0 += w
            xt_tiles = {}

            def emit_B_dma(j):
                c0, w = n_chunks[j]
                xt = xsp.tile([128, KC * 512], f8, tag="xt")
                nc.sync.dma_start(
                    out=xt[:, 0:KC * w].rearrange("p (k n) -> p k n", k=KC),
                    in_=XT_d.ap().rearrange(
                        "(k p) n -> p k n", p=128)[:, :, c0:c0 + w])
                xt_tiles[j] = xt

            # ---- fused LSTM step ------------------------------------
            def emit_C(t, mid_act=None, end_act=None):
                c0, w = n_chunks[t // 8]
                xt3 = xt_tiles[t // 8][:, 0:KC * w].rearrange(
                    "p (k n) -> p k n", k=KC)
                toff = t * B - c0
                if t == 0:
                    hrhs = xTb3
                    hoff = 0
                else:
                    hrhs = hs3
                    hoff = (t - 1) * B
                # ih matmuls first (no dependency on the h chain), then
                # hh+bias; g/o half (m 8..15) first so its tanh overlaps
                # the i/f half's matmuls.
                ps1 = psC.tile([128, 512], f32, tag="ps1")
                ps0 = psC.tile([128, 512], f32, tag="ps0")
                order = list(range(8, 16)) + list(range(8))
                for m in order:
                    ps = ps0 if m < 8 else ps1
                    col = (m % 8) * B
                    for pr in range(2):
                        nc.tensor.matmul(
                            ps[:, col:col + B],
                            Wih3[:, 2 * pr:2 * pr + 2,
                                 m * 128:(m + 1) * 128],
                            xt3[:, 2 * pr:2 * pr + 2, toff:toff + B],
                            start=(pr == 0), stop=False, perf_mode=DR)
                for m in order:
                    ps = ps0 if m < 8 else ps1
                    col = (m % 8) * B
                    for pr in range(2):
                        nc.tensor.matmul(
                            ps[:, col:col + B],
                            Whh3[:, 2 * pr:2 * pr + 2,
                                 m * 128:(m + 1) * 128],
                            hrhs[:, 2 * pr:2 * pr + 2, hoff:hoff + B],
                            start=False, stop=False, perf_mode=DR)
                    nc.tensor.matmul(
                        ps[:, col:col + B],
                        biasb[:, m * 128:(m + 1) * 128],
                        ones[:, 0:B], start=False, stop=True)
                # all-tanh gates: tanh(x/512) of x256-scaled = tanh(g/2)
                s1 = gsp.tile([128, 512], f32, tag="s1")
                nc.scalar.activation(out=s1[:, :], in_=ps1[:, :],
                                     func=AF.Tanh, scale=1.0 / 512)
                tg = s1[:, 0:256]
                to = s1[:, 256:512]
                to8 = gsp.tile([128, 256], f32, tag="to8")
                nc.vector.tensor_scalar(
                    out=to8[:, :], in0=to, scalar1=8.0, scalar2=8.0,
                    op0=AluOpType.mult, op1=AluOpType.add)
                s0 = gsp.tile([128, 512], f32, tag="s0")
                nc.scalar.activation(out=s0[:, :], in_=ps0[:, :],
                                     func=AF.Tanh, scale=1.0 / 512)
                ti = s0[:, 0:256]
                tf = s0[:, 256:512]
                # deferred fc exp lands here: it fills the ACT gap while
                # DVE runs the c-state chain
                if mid_act is not None:
                    mid_act()
                # state is c2 = 2*c:  c2' = 0.5*(1+tf)*c2 + (1+ti)*tg
                u = gsp.tile([128, 256], f32, tag="u")
                nc.vector.scalar_tensor_tensor(
                    out=u[:, :], in0=tf, scalar=1.0,
                    in1=cT[:, :], op0=AluOpType.add, op1=AluOpType.mult)
                v = gsp.tile([128, 256], f32, tag="v")
                nc.vector.scalar_tensor_tensor(
                    out=v[:, :], in0=ti, scalar=1.0,
                    in1=tg, op0=AluOpType.add, op1=AluOpType.mult)
                nc.vector.scalar_tensor_tensor(
                    out=cT[:, :], in0=u[:, :], scalar=0.5,
                    in1=v[:, :], op0=AluOpType.mult, op1=AluOpType.add)
                th = gsp.tile([128, 256], f32, tag="th")
                nc.scalar.activation(out=th[:, :], in_=cT[:, :],
                                     func=AF.Tanh, scale=0.5)
                # h*16 = (8 + 8*to) * tanh(c), written as x16-scaled fp8
                hout = hs3[:, :, t * B:(t + 1) * B]
                nc.vector.tensor_tensor(out=hout, in0=to8[:, :],
                                        in1=th[:, :], op=AluOpType.mult)
                if end_act is not None:
                    end_act()
                # stage this step's h into the k-major fc layout (gpsimd:
                # off the critical chain, Pool engine is otherwise idle)
                half = (t % 2) * B
                nc.gpsimd.tensor_copy(
                    out=hsTk4[:, t // 2, :, half:half + B],
                    in_=hout)

            # ---- phase D unit: rows [128m,128m+mw) x 1024 vocab -----
            def emit_D_mm(m, q):
                mw = min(128, R - m * 128)
                uw = min(NV, VS - q * NV)
                ps = psD.tile([128, NV], f32, tag="dps")
                for j in range((uw + 511) // 512):
                    n0 = q * NV + j * 512
                    nw = min(512, VS - n0)
                    for pr in range(2):
                        nc.tensor.matmul(
                            ps[0:mw, j * 512:j * 512 + nw],
                            hsTk4[:, m, 2 * pr:2 * pr + 2, 0:mw],
                            fcW3[:, 2 * pr:2 * pr + 2, n0:n0 + nw],
                            start=(pr == 0), stop=False, perf_mode=DR)
                    nc.tensor.matmul(
                        ps[0:mw, j * 512:j * 512 + nw], ones[:, 0:mw],
                        fcb[:, n0:n0 + nw], start=False, stop=True)
                return (ps, m, q, mw, uw)

            def emit_D_exp(pend):
                # low scheduler priority: the exp is filler work -- never
                # let it delay the LSTM chain's tanh ops on ACT
                ps, m, q, mw, uw = pend
                eo = fsp.tile([128, NV], bf16, tag="eo")
                with tc.high_priority(offset=-CFG_EOPRI):
                    nc.scalar.activation(
                        out=eo[0:mw, 0:uw], in_=ps[0:mw, 0:uw], func=AF.Exp,
                        scale=1.0 / 256,
                        accum_out=S_all[0:mw, m * NQ + q:m * NQ + q + 1])

            # ---- interleaved emission -------------------------------
            emit_B_dma(0)
            emit_B_dma(1)
            for k in range(KC):
                nc.sync.dma_start(
                    out=fcW[:, k * VS:(k + 1) * VS],
                    in_=fcWT_d[k * 128:(k + 1) * 128, :])
            nc.sync.dma_start(out=fcb[:, :], in_=fcb_d[:, :])
            d_queue = [(m, q) for m in range(MC_R) for q in range(NQ)]
            d_next = 0
            pending = []
            for t in range(TP1):
                # D matmul units first: PE runs them during the previous
                # step's tanh/c chain; their exps are deferred into this
                # step's ACT gaps (mid_act / end_act)
                # 2 extra steps of slack past the data-ready edge so the
                # fc matmuls never launch right behind the hsTk copy
                m_ready = (t - 4) // 2 if t >= 4 else -1
                new_units = []
                while len(new_units) < CFG_DBUDGET and d_next < len(d_queue):
                    m, q = d_queue[d_next]
                    if m > m_ready:
                        break
                    new_units.append(emit_D_mm(m, q))
                    d_next += 1
                if t % 8 == 0:
                    j = t // 8 + 2
                    if j < len(n_chunks):
                        emit_B_dma(j)
                exps = list(pending)
                pending = new_units

                def mid():
                    if exps:
                        emit_D_exp(exps.pop(0))

                def end():
                    while exps:
                        emit_D_exp(exps.pop(0))

                emit_C(t, mid_act=mid, end_act=end)
            while d_next < len(d_queue) or pending:
                for p in pending:
                    emit_D_exp(p)
                pending = []
                n_emit = 0
                while n_emit < CFG_DBUDGET and d_next < len(d_queue):
                    m, q = d_queue[d_next]
                    pending.append(emit_D_mm(m, q))
                    d_next += 1
                    n_emit += 1

            nc.sync.dma_start(out=hs_d[:, :], in_=hsT[:, :])
            S_fin = gp.tile([128, MC_R], f32)
            nc.vector.reduce_sum(
                out=S_fin[:, :],
                in_=S_all[:, :].rearrange("p (m n) -> p m n", n=NQ),
                axis=mybir.AxisListType.X)
            nc.sync.dma_start(out=S_d[:, :], in_=S_fin[:, :])

    nc.compile()
    return nc


def _get_built():
    global _BUILT
    if _BUILT is None:
        _BUILT = _build()
    return _BUILT


def _q8(a):
    return np.clip(a, -240.0, 240.0).astype(mld.float8_e4m3)


def prep_in_maps(x, labels, emb, W_ih, W_hh, b_ih, b_hh, fc_W, fc_b):
    lab = labels.astype(np.int64)
    inputs = np.concatenate(
        [np.full((B, 1), START_IDX, np.int64), lab], axis=1)      # [B, 51]
    targets = np.concatenate(
        [lab, np.full((B, 1), STOP_IDX, np.int64)], axis=1)       # [B, 51]
    idx = inputs.T.reshape(-1)      # [3264] t-major
    tgt = targets.T.reshape(-1)

    # unified tanh(x/512): g-gate rows (the tanh gate) carry half scale
    gsc = np.ones((G,), np.float32)
    gsc[2 * H:3 * H] = 2.0
    base = {
        "xTb": _q8(np.ascontiguousarray(x.T) * SCL),
        "xTf": (np.ascontiguousarray(x.T) * 2.0).astype(np.float32),
        "XT": _q8(np.ascontiguousarray(emb[idx].T) * SCL),
        "WihT": _q8(np.ascontiguousarray((W_ih * gsc[:, None]).T) * SCL),
        "WhhT": _q8(np.ascontiguousarray((W_hh * gsc[:, None]).T) * SCL),
        "biasb": ((b_ih + b_hh) * gsc * 256.0)[None, :].astype(mld.bfloat16),
    }
    in_maps = []
    for c in range(NC):
        sh = slice(c * VS, (c + 1) * VS)
        in_maps.append(dict(
            base,
            fcWT=_q8(np.ascontiguousarray(fc_W[sh].T) * SCL),
            fcb=(fc_b[sh][None, :] * 256.0).astype(mld.bfloat16)))
    return in_maps, tgt


def combine(results, tgt, fc_W, fc_b):
    S_rows = np.zeros(R, np.float64)
    for c in range(NC):
        S_rows += np.asarray(
            results[c]["S"], np.float64).T.reshape(-1)[:R]
    hs0 = np.asarray(results[0]["hs"]).astype(np.float32) / SCL   # [128, R*4]
    hs_rows = hs0.reshape(128, R, KC).transpose(1, 2, 0).reshape(R, H)
    Wt = fc_W[tgt].astype(mld.bfloat16).astype(np.float32)        # [3264, 512]
    tgt_dot = (hs_rows * Wt).sum(1, dtype=np.float32)
    nll = np.log(S_rows) - (tgt_dot.astype(np.float64) + fc_b[tgt])
    return np.float32(nll.sum() / B)


def kernel(x, labels, emb, W_ih, W_hh, b_ih, b_hh, fc_W, fc_b):
    from concourse.bass_utils import run_bass_kernel_spmd

    x = np.asarray(x, np.float32)
    emb = np.asarray(emb, np.float32)
    W_ih = np.asarray(W_ih, np.float32)
    W_hh = np.asarray(W_hh, np.float32)
    b_ih = np.asarray(b_ih, np.float32)
    b_hh = np.asarray(b_hh, np.float32)
    fc_W = np.asarray(fc_W, np.float32)
    fc_b = np.asarray(fc_b, np.float32)

    in_maps, tgt = prep_in_maps(x, np.asarray(labels), emb, W_ih, W_hh,
                                b_ih, b_hh, fc_W, fc_b)
    nc = _get_built()
    res = run_bass_kernel_spmd(nc, in_maps, core_ids=list(range(NC)))
    return combine(res.results, tgt, fc_W, fc_b)
